# revision 1
# baseline (speedup 1.0000x reference)
"""Trainium2 Bass kernel for windowed (sink/ring-buffer) self-attention with RoPE.

Contract: kernel(**inputs) takes FULL unsharded inputs (as produced by the
problem's setup_inputs) and returns the FULL output [B, L, n, d].

Sharding: 12 heads x 1440 queries are split across 8 NeuronCores as
1.5 "head-units" per core: each core owns one full head (1440 queries) plus
half of a head shared with its pair core (720 queries). All cores run the
same SPMD program on differently-sliced inputs.

Device program (per core):
  - RoPE applied on-chip to q and the new k block (4 tensor ops per block,
    using host-precomputed cos/sin tables in a de-interleaved d-layout that
    turns the rotation into plain elementwise ops; the d-permutation cancels
    inside the QK^T contraction).
  - S^T = ka^T q computed in [kv, q] orientation (fp32r matmuls), exp on
    ScalarE straight out of PSUM, then OT = va^T P and softmax denominators
    accumulated in PSUM; final transpose back to [q, d] on TensorE with a
    per-partition reciprocal scale.
"""

import math

import numpy as np

P = 128
THETA = 10000.0
LOCAL_ATTN_SIZE = 15
SINK_SIZE = 1

QBLK = 768          # q columns per pass (2 psum banks: 512 + 256 chunks)
CHUNKS = ((0, 512), (512, 256))

_BUILD_CACHE = {}


# ----------------------------------------------------------------------------
# host-side planning (mirrors the reference's python-int index logic)
# ----------------------------------------------------------------------------

def _plan_cache_segments(current_start, global_end_index, local_end_index,
                         num_new, cache_len, frame_seqlen):
    """Return (segments, local_end, kv_start): list of (lo, hi) slices of the
    ORIGINAL cache arrays that make up the pre-new-token part of the attention
    window, mirroring reference.py's roll/evict logic."""
    current_end = current_start + num_new
    sink_tokens = SINK_SIZE * frame_seqlen
    max_attn = LOCAL_ATTN_SIZE * frame_seqlen
    if current_end > global_end_index and num_new + local_end_index > cache_len:
        n_evict = num_new + local_end_index - cache_len
        n_roll = local_end_index - n_evict - sink_tokens
        local_end = local_end_index + current_end - global_end_index - n_evict
        roll_lo, roll_hi = sink_tokens, sink_tokens + n_roll

        def old_index(i):
            return i + n_evict if roll_lo <= i < roll_hi else i
    else:
        local_end = local_end_index + current_end - global_end_index
        n_evict = 0

        def old_index(i):
            return i

    local_start = local_end - num_new
    kv_start = max(0, local_end - max_attn)
    # contiguous segments of old_index over [kv_start, local_start)
    segs = []
    i = kv_start
    while i < local_start:
        lo = old_index(i)
        j = i
        while j + 1 < local_start and old_index(j + 1) == old_index(j) + 1:
            j += 1
        segs.append((lo, lo + (j - i + 1)))
        i = j + 1
    return segs, local_end, kv_start


def _rope_cos_sin(L, d, grid_h, grid_w, start_frame):
    """cos/sin angle tables [L, d//2] matching reference make_freqs/rope_apply."""
    c = d // 2
    d1 = d - 4 * (d // 6)
    d2 = 2 * (d // 6)
    inv1 = THETA ** (-(np.arange(0, d1, 2, dtype=np.float32) / np.float32(d1)))
    inv2 = THETA ** (-(np.arange(0, d2, 2, dtype=np.float32) / np.float32(d2)))
    inv3 = inv2
    hw = grid_h * grid_w
    pos = np.arange(L)
    f = pos // hw + start_frame
    hh = (pos % hw) // grid_w
    ww = pos % grid_w
    ang = np.concatenate([
        f[:, None].astype(np.float32) * inv1[None, :],
        hh[:, None].astype(np.float32) * inv2[None, :],
        ww[:, None].astype(np.float32) * inv3[None, :],
    ], axis=1)
    assert ang.shape == (L, c)
    return np.cos(ang).astype(np.float32), np.sin(ang).astype(np.float32)


# ----------------------------------------------------------------------------
# device program
# ----------------------------------------------------------------------------

def _build_program(L, d, n_cache, n_kv):
    """Build the SPMD Bass program for one core.

    L: new-token count (1440); d: head dim (128); n_cache: cache rows in the
    window (9360); n_kv: total kv rows (10800)."""
    import concourse.bass as bass
    import concourse.mybir as mybir
    import concourse.tile as tile
    from concourse import bacc

    f32 = mybir.dt.float32
    f32r = mybir.dt.float32r
    Exp = mybir.ActivationFunctionType.Exp

    n_kv_pad = ((n_kv + P - 1) // P) * P          # 10880
    KT = n_kv_pad // P                            # 85 k-tiles
    last_valid = n_kv - (KT - 1) * P              # 48 valid rows in last k-tile
    scale = 1.0 / math.sqrt(d)

    # q-block layout within the per-core q tensor: [A (1440->1536 pad) | B (720->768 pad)]
    ablk = ((L + QBLK - 1) // QBLK) * QBLK        # 1536
    QT_N = ablk + QBLK                            # 2304
    # passes: (ka/va slot, q column offset)
    passes = [("a", 0), ("a", QBLK), ("b", ablk)]

    nc = bacc.Bacc(None, target_bir_lowering=False)

    qt_d = nc.dram_tensor("qt", [P, QT_N], f32r, kind="ExternalInput")
    qts_d = nc.dram_tensor("qts", [P, QT_N], f32, kind="ExternalInput")
    cosq_d = nc.dram_tensor("cosq", [P, QT_N], f32, kind="ExternalInput")
    sinq_d = nc.dram_tensor("sinq", [P, QT_N], f32, kind="ExternalInput")
    kt_d = {s: nc.dram_tensor(f"kt{s}", [P, n_kv_pad - n_cache], f32r,
                              kind="ExternalInput") for s in "ab"}
    kts_d = {s: nc.dram_tensor(f"kts{s}", [P, L], f32,
                               kind="ExternalInput") for s in "ab"}
    kc_d = {s: nc.dram_tensor(f"kc{s}", [P, n_cache], f32r,
                              kind="ExternalInput") for s in "ab"}
    va_d = {s: nc.dram_tensor(f"va{s}", [n_kv_pad, d], f32r,
                              kind="ExternalInput") for s in "ab"}
    # [128, 256]: cols 0:128 all-ones matrix, cols 128:256 rows<last_valid ones
    ones_d = nc.dram_tensor("onesm", [P, 2 * P], f32r, kind="ExternalInput")
    ident_d = nc.dram_tensor("ident", [P, P], f32, kind="ExternalInput")
    out_d = nc.dram_tensor("o", [QT_N, d], f32, kind="ExternalOutput")

    with tile.TileContext(nc) as tc:
        with tc.tile_pool(name="big", bufs=1) as big, \
             tc.tile_pool(name="work", bufs=2) as work, \
             tc.tile_pool(name="psum", bufs=1, space="PSUM") as psum:

            ident = big.tile([P, P], f32, tag="ident", name="ident")
            nc.sync.dma_start(ident[:], ident_d[:])
            onesm = big.tile([P, 2 * P], f32r, tag="onesm", name="onesm")
            nc.sync.dma_start(onesm[:], ones_d[:])

            cosq = big.tile([P, QT_N], f32, tag="cosq", name="cosq")
            sinq = big.tile([P, QT_N], f32, tag="sinq", name="sinq")

            rq = big.tile([P, QT_N], f32r, tag="rq", name="rq")
            ka = big.tile([P, n_kv_pad], f32r, tag="ka", name="ka")
            va = big.tile([P, n_kv_pad], f32r, tag="va", name="va")

            def rope(dst_f32r, src_f32r, swap_f32, n_cols, tab_off):
                """dst = rope(src) where swap_f32 holds the half-swapped copy
                (host-built); all operands lane-aligned [P, n_cols]. Runs in
                <=QBLK column chunks so downstream matmuls unblock early."""
                for c0 in range(0, n_cols, QBLK):
                    w = min(QBLK, n_cols - c0)
                    src = src_f32r[:, c0:c0 + w].bitcast(f32)
                    C = cosq[:, tab_off + c0:tab_off + c0 + w]
                    S = sinq[:, tab_off + c0:tab_off + c0 + w]
                    t1 = work.tile([P, w], f32, tag="ropet1", name="ropet1")
                    t2 = work.tile([P, w], f32, tag="ropet2", name="ropet2")
                    nc.vector.tensor_mul(t1[:, :], swap_f32[:, c0:c0 + w], S)
                    nc.vector.tensor_mul(t2[:, :], src, C)
                    nc.vector.tensor_add(dst_f32r[:, c0:c0 + w],
                                         t2[:, :].bitcast(f32r),
                                         t1[:, :].bitcast(f32r))

            # --- q load + rope (both blocks); x lands in rq, swap staged ---
            # chunked so the first rope chunk (and first matmul) starts early
            qsw = work.tile([P, QT_N], f32, tag="swstage", bufs=1, name="qsw")
            for c0 in range(0, QT_N, QBLK):
                c1 = c0 + QBLK
                nc.sync.dma_start(rq[:, c0:c1], qt_d[:, c0:c1])
                nc.sync.dma_start(qsw[:, c0:c1], qts_d[:, c0:c1])
                nc.sync.dma_start(cosq[:, c0:c1], cosq_d[:, c0:c1])
                nc.sync.dma_start(sinq[:, c0:c1], sinq_d[:, c0:c1])
            rope(rq[:, 0:ablk], rq[:, 0:ablk], qsw[:, 0:ablk], ablk, 0)
            rope(rq[:, ablk:QT_N], rq[:, ablk:QT_N], qsw[:, ablk:QT_N], QBLK,
                 ablk)

            def load_kv_slot(s):
                """DMA cache keys + new keys + values for slot s; rope new keys."""
                nch = 8
                ncols = n_cache // nch
                for cidx in range(nch):
                    lo = cidx * ncols
                    hi = n_cache if cidx == nch - 1 else lo + ncols
                    nc.sync.dma_start(ka[:, lo:hi], kc_d[s][:, lo:hi])
                nc.sync.dma_start(ka[:, n_cache:n_kv_pad], kt_d[s][:])
                ksw = work.tile([P, L], f32, tag="swstage", bufs=1,
                                name=f"ksw{s}")
                nc.sync.dma_start(ksw[:], kts_d[s][:])
                rope(ka[:, n_cache:n_cache + L], ka[:, n_cache:n_cache + L],
                     ksw[:, :], L, 0)
                # values: [n_kv_pad, d] rows -> [P, KT*d] tiles
                src = va_d[s][:].rearrange("(t p) d -> p t d", p=P)
                dst = va[:].rearrange("p (t d) -> p t d", d=d)
                qtr = KT // 8
                for cidx in range(8):
                    t0 = cidx * qtr
                    t1_ = KT if cidx == 7 else (cidx + 1) * qtr
                    nc.sync.dma_start(dst[:, t0:t1_, :], src[:, t0:t1_, :])

            load_kv_slot("a")

            # B-pass DVE softmax-denominator accumulators (ping-pong)
            sacc = [big.tile([P, QBLK], f32, tag=f"sacc{i}", name=f"sacc{i}")
                    for i in range(2)]

            def run_pass(pidx, slot, q0, dve_frac=4):
                """One 768-wide q pass. Software-pipelined one k-tile deep:
                S^T(kt+1) is issued before AV/sums(kt) so TensorE never stalls
                on exp(kt). Softmax denominators: k-tiles with kt % 5 <
                dve_frac accumulate on DVE (ping-pong adds), the rest via
                ones-matmul on TensorE; both fold into sums_ps at pass end."""
                ot_ps = psum.tile([P, QBLK], f32, tag="ot", name=f"ot{pidx}")
                sums_ps = psum.tile([P, QBLK], f32, tag="sums", name=f"sums{pidx}")

                pts = {}
                state = dict(pe_first=True, n_dve=0)

                def st_mm(kt):
                    ksl = ka[:, kt * P:(kt + 1) * P]
                    sc = psum.tile([P, QBLK], f32, tag="sc", bufs=2,
                                   name=f"sc{pidx}_{kt}")
                    for (co, cw) in CHUNKS:
                        nc.tensor.matmul(sc[:, co:co + cw], ksl,
                                         rq[:, q0 + co:q0 + co + cw],
                                         start=True, stop=True)
                    pt = work.tile([P, QBLK], f32r, tag="pt", bufs=4,
                                   name=f"pt{pidx}_{kt}")
                    nc.scalar.activation(pt[:], sc[:, :], Exp, scale=scale)
                    pts[kt] = pt

                def av_sums(kt):
                    pt = pts.pop(kt)
                    vsl = va[:, kt * d:(kt + 1) * d]
                    first, last = kt == 0, kt == KT - 1
                    for (co, cw) in CHUNKS:
                        nc.tensor.matmul(ot_ps[:, co:co + cw], vsl,
                                         pt[:, co:co + cw],
                                         start=first, stop=last)
                    on_dve = kt % 5 < dve_frac and kt != KT - 1
                    if on_dve:
                        n = state["n_dve"]
                        if n == 0:
                            nc.vector.tensor_copy(sacc[0][:], pt[:].bitcast(f32))
                        else:
                            nc.vector.tensor_add(sacc[n % 2][:],
                                                 sacc[(n + 1) % 2][:],
                                                 pt[:].bitcast(f32))
                        state["n_dve"] = n + 1
                    else:
                        onemat = (onesm[:, P:2 * P] if kt == KT - 1
                                  else onesm[:, 0:P])
                        for (co, cw) in CHUNKS:
                            nc.tensor.matmul(sums_ps[:, co:co + cw], onemat,
                                             pt[:, co:co + cw],
                                             start=state["pe_first"], stop=False)
                        state["pe_first"] = False

                st_mm(0)
                for kt in range(KT):
                    if kt + 1 < KT:
                        st_mm(kt + 1)
                    av_sums(kt)
                # fold the DVE accumulator into sums_ps
                saccr = work.tile([P, QBLK], f32r, tag="saccr", bufs=1,
                                  name=f"saccr{pidx}")
                nc.vector.tensor_copy(saccr[:],
                                      sacc[(state["n_dve"] + 1) % 2][:])
                for (co, cw) in CHUNKS:
                    nc.tensor.matmul(sums_ps[:, co:co + cw], onesm[:, 0:P],
                                     saccr[:, co:co + cw],
                                     start=False, stop=True)

                # ---- drain: transpose + normalize + store ----
                ot_sb = work.tile([P, QBLK], f32, tag="otsb", name=f"otsb{pidx}")
                nc.vector.tensor_copy(ot_sb[:], ot_ps[:, :])
                # sums rows are all identical; keep lane 0
                s_sb = work.tile([1, QBLK], f32, tag="ssb", name=f"ssb{pidx}")
                nc.vector.tensor_copy(s_sb[0:1, :], sums_ps[0:1, :])
                for j in range(QBLK // P):
                    tp = psum.tile([P, P + 1], f32, tag="sc", bufs=2,
                                   name=f"tp{pidx}_{j}")
                    nc.tensor.transpose(tp[:, 0:P],
                                        ot_sb[:, j * P:(j + 1) * P], ident[:])
                    nc.tensor.transpose(tp[:, P:P + 1],
                                        s_sb[0:1, j * P:(j + 1) * P],
                                        ident[0:1, 0:1])
                    r_sb = work.tile([P, 1], f32, tag="rsb", name=f"rsb{pidx}_{j}")
                    nc.vector.reciprocal(r_sb[:], tp[:, P:P + 1])
                    o_sb = work.tile([P, d], f32, tag="osb", bufs=3,
                                     name=f"osb{pidx}_{j}")
                    nc.vector.tensor_scalar_mul(o_sb[:], tp[:, 0:P], r_sb[:])
                    row0 = q0 + j * P
                    nc.sync.dma_start(out_d[row0:row0 + P, :], o_sb[:])

            run_pass(0, "a", 0)
            run_pass(1, "a", QBLK)
            load_kv_slot("b")
            run_pass(2, "b", ablk)

    nc.finalize()
    meta = dict(QT_N=QT_N, ablk=ablk, n_kv_pad=n_kv_pad, last_valid=last_valid)
    return nc, meta


# ----------------------------------------------------------------------------
# host wrapper
# ----------------------------------------------------------------------------

def kernel(q, k, v, k_cache, v_cache, current_start, global_end_index,
           local_end_index, grid_f, grid_h, grid_w):
    from concourse.bass_utils import run_bass_kernel_spmd

    q = np.asarray(q, dtype=np.float32)
    k = np.asarray(k, dtype=np.float32)
    v = np.asarray(v, dtype=np.float32)
    k_cache = np.asarray(k_cache, dtype=np.float32)
    v_cache = np.asarray(v_cache, dtype=np.float32)
    current_start = int(current_start)
    global_end_index = int(global_end_index)
    local_end_index = int(local_end_index)
    grid_h, grid_w = int(grid_h), int(grid_w)

    B, L, n_heads, d = q.shape
    cache_len = k_cache.shape[1]
    frame_seqlen = grid_h * grid_w
    start_frame = current_start // frame_seqlen

    segs, local_end, kv_start = _plan_cache_segments(
        current_start, global_end_index, local_end_index, L, cache_len,
        frame_seqlen)
    n_cache = sum(hi - lo for lo, hi in segs)
    n_kv = n_cache + L

    key = (L, d, n_cache, n_kv)
    if key not in _BUILD_CACHE:
        _BUILD_CACHE[key] = _build_program(L, d, n_cache, n_kv)
    nc, meta = _BUILD_CACHE[key]
    QT_N, ablk, n_kv_pad = meta["QT_N"], meta["ablk"], meta["n_kv_pad"]
    last_valid = meta["last_valid"]

    # gather the cache window once (numpy)
    kc_full = np.concatenate([k_cache[0, lo:hi] for lo, hi in segs], axis=0)
    vc_full = np.concatenate([v_cache[0, lo:hi] for lo, hi in segs], axis=0)

    cos_t, sin_t = _rope_cos_sin(L, d, grid_h, grid_w, start_frame)  # [L, 64]
    H = d // 2
    perm = np.concatenate([np.arange(0, d, 2), np.arange(1, d, 2)])


    onesm = np.zeros((P, 2 * P), dtype=np.float32)
    onesm[:, 0:P] = 1.0
    onesm[0:last_valid, P:2 * P] = 1.0
    ident = np.eye(P, dtype=np.float32)

    perm_swap = np.concatenate([np.arange(1, d, 2), np.arange(0, d, 2)])

    def dei_T(x):  # [rows, d] -> de-interleaved transpose [d, rows]
        return np.ascontiguousarray(x.T[perm])

    def dei_T_swap(x):  # half-swapped variant: [odds; evens]
        return np.ascontiguousarray(x.T[perm_swap])

    half = L // 2
    n_pairs = n_heads // 3
    assert n_heads % 3 == 0 and n_pairs * 2 == 8, "sharding expects 12 heads/8 cores"

    in_maps = []
    core_heads = []
    for c in range(8):
        p, s = c // 2, c % 2
        headA = 3 * p if s == 0 else 3 * p + 2
        headB = 3 * p + 1
        qsl = slice(0, half) if s == 0 else slice(half, L)
        core_heads.append((headA, headB, qsl))

        cosq = np.ones((P, QT_N), dtype=np.float32)
        sinq = np.zeros((P, QT_N), dtype=np.float32)
        for (c0, tab) in ((0, slice(0, L)), (ablk, qsl)):
            ct, st = cos_t[tab].T, sin_t[tab].T
            w = ct.shape[1]
            cosq[0:H, c0:c0 + w] = ct
            cosq[H:P, c0:c0 + w] = ct
            sinq[0:H, c0:c0 + w] = -st
            sinq[H:P, c0:c0 + w] = st

        qt = np.zeros((P, QT_N), dtype=np.float32)
        qt[:, 0:L] = dei_T(q[0, :, headA, :])
        qt[:, ablk:ablk + half] = dei_T(q[0, qsl, headB, :])
        qts = np.zeros((P, QT_N), dtype=np.float32)
        qts[:, 0:L] = dei_T_swap(q[0, :, headA, :])
        qts[:, ablk:ablk + half] = dei_T_swap(q[0, qsl, headB, :])

        im = {"qt": qt, "qts": qts, "cosq": cosq, "sinq": sinq,
              "onesm": onesm, "ident": ident}
        for tag, h in (("a", headA), ("b", headB)):
            ktn = np.zeros((P, n_kv_pad - n_cache), dtype=np.float32)
            ktn[:, 0:L] = dei_T(k[0, :, h, :])
            im[f"kt{tag}"] = ktn
            im[f"kts{tag}"] = dei_T_swap(k[0, :, h, :])
            im[f"kc{tag}"] = dei_T(kc_full[:, h, :])
            vaa = np.zeros((n_kv_pad, d), dtype=np.float32)
            vaa[0:n_cache] = vc_full[:, h, :]
            vaa[n_cache:n_cache + L] = v[0, :, h, :]
            im[f"va{tag}"] = vaa
        in_maps.append(im)

    res = run_bass_kernel_spmd(nc, in_maps, core_ids=list(range(8)))

    out = np.empty((B, L, n_heads, d), dtype=np.float32)
    for c in range(8):
        headA, headB, qsl = core_heads[c]
        o = res.results[c]["o"]
        out[0, :, headA, :] = o[0:L]
        out[0, qsl, headB, :] = o[ablk:ablk + half]
    return out



# revision 4
# speedup vs baseline: 1.2321x; 1.2321x over previous
"""Trainium2 Bass kernel for windowed (sink/ring-buffer) self-attention with RoPE.

Contract: kernel(**inputs) takes FULL unsharded inputs (as produced by the
problem's setup_inputs) and returns the FULL output [B, L, n, d].

Sharding: 12 heads x 1440 queries are split across 8 NeuronCores as
1.5 "head-units" per core: each core owns one full head (1440 queries) plus
half of a head shared with its pair core (720 queries). All cores run the
same SPMD program on differently-sliced inputs.

Device program (per core), v2 (fp16 datapath):
  - All matmul operands fp16 (q/k roped on-chip into fp16, v fp16 from host,
    exp output fp16). PSUM accumulation stays fp32.
  - Both KV head-slots live in SBUF simultaneously (fp16 halves the
    footprint), so all DMAs are issued up front and no pass stalls on a
    cache reload.
  - 3 uniform passes of 720 q columns over 85 kv tiles. Scores for two kv
    tiles are computed into one [128, 1440] PSUM tile so a single ScalarE
    activation (exp) covers both, halving per-instruction overhead on the
    bottleneck engine.
  - Softmax denominators: DVE accumulates exp tiles in fp16 (2x mode); the
    padded kv lanes of the last (ragged) tile are killed inside the exp
    itself via a per-partition bias of -30. The final partition-reduction is
    one ones-matmul on PE; output is normalized BEFORE the transpose so the
    transposed result DMAs straight from PSUM to DRAM.
"""

import math

import numpy as np

P = 128
THETA = 10000.0
LOCAL_ATTN_SIZE = 15
SINK_SIZE = 1

QBLK = 720          # q columns per pass (3 uniform passes over 2160 cols)

_BUILD_CACHE = {}


# ----------------------------------------------------------------------------
# host-side planning (mirrors the reference's python-int index logic)
# ----------------------------------------------------------------------------

def _plan_cache_segments(current_start, global_end_index, local_end_index,
                         num_new, cache_len, frame_seqlen):
    """Return (segments, local_end, kv_start): list of (lo, hi) slices of the
    ORIGINAL cache arrays that make up the pre-new-token part of the attention
    window, mirroring reference.py's roll/evict logic."""
    current_end = current_start + num_new
    sink_tokens = SINK_SIZE * frame_seqlen
    max_attn = LOCAL_ATTN_SIZE * frame_seqlen
    if current_end > global_end_index and num_new + local_end_index > cache_len:
        n_evict = num_new + local_end_index - cache_len
        n_roll = local_end_index - n_evict - sink_tokens
        local_end = local_end_index + current_end - global_end_index - n_evict
        roll_lo, roll_hi = sink_tokens, sink_tokens + n_roll

        def old_index(i):
            return i + n_evict if roll_lo <= i < roll_hi else i
    else:
        local_end = local_end_index + current_end - global_end_index
        n_evict = 0

        def old_index(i):
            return i

    local_start = local_end - num_new
    kv_start = max(0, local_end - max_attn)
    # contiguous segments of old_index over [kv_start, local_start)
    segs = []
    i = kv_start
    while i < local_start:
        lo = old_index(i)
        j = i
        while j + 1 < local_start and old_index(j + 1) == old_index(j) + 1:
            j += 1
        segs.append((lo, lo + (j - i + 1)))
        i = j + 1
    return segs, local_end, kv_start


def _rope_cos_sin(L, d, grid_h, grid_w, start_frame):
    """cos/sin angle tables [L, d//2] matching reference make_freqs/rope_apply."""
    c = d // 2
    d1 = d - 4 * (d // 6)
    d2 = 2 * (d // 6)
    inv1 = THETA ** (-(np.arange(0, d1, 2, dtype=np.float32) / np.float32(d1)))
    inv2 = THETA ** (-(np.arange(0, d2, 2, dtype=np.float32) / np.float32(d2)))
    inv3 = inv2
    hw = grid_h * grid_w
    pos = np.arange(L)
    f = pos // hw + start_frame
    hh = (pos % hw) // grid_w
    ww = pos % grid_w
    ang = np.concatenate([
        f[:, None].astype(np.float32) * inv1[None, :],
        hh[:, None].astype(np.float32) * inv2[None, :],
        ww[:, None].astype(np.float32) * inv3[None, :],
    ], axis=1)
    assert ang.shape == (L, c)
    return np.cos(ang).astype(np.float32), np.sin(ang).astype(np.float32)


# ----------------------------------------------------------------------------
# device program
# ----------------------------------------------------------------------------

def _build_program(L, d, n_cache, n_kv):
    """Build the SPMD Bass program for one core.

    L: new-token count (1440); d: head dim (128); n_cache: cache rows in the
    window (9360); n_kv: total kv rows (10800)."""
    import concourse.bass as bass
    import concourse.mybir as mybir
    import concourse.tile as tile
    from concourse import bacc

    f16 = mybir.dt.float16
    f32 = mybir.dt.float32
    Exp = mybir.ActivationFunctionType.Exp

    n_kv_pad = ((n_kv + P - 1) // P) * P          # 10880
    KT = n_kv_pad // P                            # 85 k-tiles
    npairs = KT // 2                              # 42 merged pairs
    kpad = n_kv_pad - n_kv                        # 80 zero-padded kv rows
    knew = n_kv_pad - n_cache                     # 1520 = new k cols + pad
    scale = 1.0 / math.sqrt(d)

    QT_N = 3 * QBLK                               # 2160 total q columns
    passes = (0, QBLK, 2 * QBLK)

    nc = bacc.Bacc(None, target_bir_lowering=False)

    qt_d = nc.dram_tensor("qt", [P, QT_N], f32, kind="ExternalInput")
    cosq_d = nc.dram_tensor("cosq", [P, QT_N], f32, kind="ExternalInput")
    sinq_d = nc.dram_tensor("sinq", [P, QT_N], f32, kind="ExternalInput")
    kt_d = {s: nc.dram_tensor(f"kt{s}", [P, knew], f32,
                              kind="ExternalInput") for s in "ab"}
    kc_d = {s: nc.dram_tensor(f"kc{s}", [P, n_cache], f16,
                              kind="ExternalInput") for s in "ab"}
    va_d = {s: nc.dram_tensor(f"va{s}", [n_kv_pad, d], f16,
                              kind="ExternalInput") for s in "ab"}
    bias_d = nc.dram_tensor("biasm", [P, 1], f32, kind="ExternalInput")
    ident_d = nc.dram_tensor("ident", [P, P], f32, kind="ExternalInput")
    ones_d = nc.dram_tensor("ones", [P, P], f16, kind="ExternalInput")
    out_d = nc.dram_tensor("o", [QT_N, d], f32, kind="ExternalOutput")

    with tile.TileContext(nc) as tc:
        with tc.tile_pool(name="big", bufs=1) as big, \
             tc.tile_pool(name="work", bufs=2) as work, \
             tc.tile_pool(name="psum", bufs=1, space="PSUM") as psum:

            ident = big.tile([P, P], f32, tag="ident", name="ident")
            onesm = big.tile([P, P], f16, tag="onesm", name="onesm")
            biasm = big.tile([P, 1], f32, tag="biasm", name="biasm")
            nc.sync.dma_start(ident[:], ident_d[:])
            nc.sync.dma_start(onesm[:], ones_d[:])
            nc.sync.dma_start(biasm[:], bias_d[:])

            cosq = big.tile([P, QT_N], f32, tag="cosq", name="cosq")
            sinq = big.tile([P, QT_N], f32, tag="sinq", name="sinq")

            rq = big.tile([P, QT_N], f16, tag="rq", name="rq")
            ka = {s: big.tile([P, n_kv_pad], f16, tag=f"ka{s}", name=f"ka{s}")
                  for s in "ab"}
            va = {s: big.tile([P, n_kv_pad], f16, tag=f"va{s}", name=f"va{s}")
                  for s in "ab"}

            def rope(dst_f16, src_f32, swap_f32, n_cols, tab_off, chunk):
                """dst = rope(src): y = src*C + swap*S with the sign folded
                into S; fp32 inputs, fp16 output, chunked so downstream
                matmuls unblock early."""
                for c0 in range(0, n_cols, chunk):
                    w = min(chunk, n_cols - c0)
                    C = cosq[:, tab_off + c0:tab_off + c0 + w]
                    S = sinq[:, tab_off + c0:tab_off + c0 + w]
                    t1 = work.tile([P, 768], f32, tag="ropet1", name="ropet1")
                    t2 = work.tile([P, 768], f32, tag="ropet2", name="ropet2")
                    nc.vector.tensor_mul(t1[:, 0:w], swap_f32[:, c0:c0 + w], S)
                    nc.vector.tensor_mul(t2[:, 0:w], src_f32[:, c0:c0 + w], C)
                    nc.vector.tensor_add(dst_f16[:, c0:c0 + w],
                                         t2[:, 0:w], t1[:, 0:w])

            # --- q load + rope; swapped copy comes from a partition-swapped
            # second DMA of the same DRAM tensor ---
            H = P // 2
            qst = work.tile([P, QT_N], f32, tag="qst", bufs=1, name="qst")
            qsw = work.tile([P, QT_N], f32, tag="qsw", bufs=1, name="qsw")
            for c0 in range(0, QT_N, QBLK):
                c1 = c0 + QBLK
                nc.sync.dma_start(qst[:, c0:c1], qt_d[:, c0:c1])
                nc.sync.dma_start(qsw[0:H, c0:c1], qt_d[H:P, c0:c1])
                nc.sync.dma_start(qsw[H:P, c0:c1], qt_d[0:H, c0:c1])
                nc.sync.dma_start(cosq[:, c0:c1], cosq_d[:, c0:c1])
                nc.sync.dma_start(sinq[:, c0:c1], sinq_d[:, c0:c1])
            rope(rq, qst, qsw, QT_N, 0, QBLK)

            def load_kv_slot(s):
                """DMA cache keys + new keys + values for slot s; rope new
                keys on-chip (table cols 0:knew; pad cols are zero)."""
                nch = 8
                ncols = n_cache // nch
                for cidx in range(nch):
                    lo = cidx * ncols
                    hi = n_cache if cidx == nch - 1 else lo + ncols
                    nc.sync.dma_start(ka[s][:, lo:hi], kc_d[s][:, lo:hi])
                kst = work.tile([P, knew], f32, tag=f"kst{s}", bufs=1,
                                name=f"kst{s}")
                ksw = work.tile([P, knew], f32, tag=f"ksw{s}", bufs=1,
                                name=f"ksw{s}")
                nc.sync.dma_start(kst[:], kt_d[s][:])
                nc.sync.dma_start(ksw[0:H, :], kt_d[s][H:P, :])
                nc.sync.dma_start(ksw[H:P, :], kt_d[s][0:H, :])
                rope(ka[s][:, n_cache:n_kv_pad], kst, ksw, knew, 0, knew // 2)
                # values: [n_kv_pad, d] rows -> [P, KT*d] tiles
                src = va_d[s][:].rearrange("(t p) d -> p t d", p=P)
                dst = va[s][:].rearrange("p (t d) -> p t d", d=d)
                qtr = KT // 8
                for cidx in range(8):
                    t0 = cidx * qtr
                    t1_ = KT if cidx == 7 else (cidx + 1) * qtr
                    nc.sync.dma_start(dst[:, t0:t1_, :], src[:, t0:t1_, :])

            load_kv_slot("a")
            load_kv_slot("b")

            # fp16 DVE softmax-denominator accumulators (ping-pong)
            sacc = [big.tile([P, QBLK], f16, tag=f"sacc{i}", name=f"sacc{i}")
                    for i in range(2)]

            def chunks_of(lo, hi):
                """Split [lo, hi) at 512-col PSUM bank boundaries."""
                cuts = [lo]
                b = (lo // 512 + 1) * 512
                while b < hi:
                    cuts.append(b)
                    b += 512
                cuts.append(hi)
                return list(zip(cuts[:-1], cuts[1:]))

            def run_pass(pidx, slot, q0):
                """One 720-wide q pass over 85 kv tiles (42 pairs + 1 single).
                Software-pipelined one pair deep."""
                kas, vas = ka[slot], va[slot]
                ot_ps = psum.tile([P, QBLK], f32, tag="ot", name=f"ot{pidx}")

                pts = {}
                state = dict(nadd=0)

                def st_unit(u):
                    """Scores for unit u into one PSUM tile + one exp."""
                    single = u == npairs
                    w = QBLK if single else 2 * QBLK
                    sc = psum.tile([P, 2 * QBLK], f32, tag="sc", bufs=2,
                                   name=f"sc{pidx}_{u}")
                    for j in range(1 if single else 2):
                        kt = 2 * u + j
                        ksl = kas[:, kt * P:(kt + 1) * P]
                        for (lo, hi) in chunks_of(j * QBLK, (j + 1) * QBLK):
                            nc.tensor.matmul(sc[:, lo:hi], ksl,
                                             rq[:, q0 + lo - j * QBLK:
                                                 q0 + hi - j * QBLK],
                                             start=True, stop=True)
                    pt = work.tile([P, 2 * QBLK], f16, tag="pt", bufs=4,
                                   name=f"pt{pidx}_{u}")
                    if single:
                        # bias kills the zero-padded kv lanes (exp ~ 0)
                        nc.scalar.activation(pt[:, 0:w], sc[:, 0:w], Exp,
                                             bias=biasm[:, 0:1], scale=scale)
                    else:
                        nc.scalar.activation(pt[:, 0:w], sc[:, 0:w], Exp,
                                             scale=scale)
                    pts[u] = pt

                def av_unit(u):
                    pt = pts.pop(u)
                    single = u == npairs
                    for j in range(1 if single else 2):
                        kt = 2 * u + j
                        vsl = vas[:, kt * d:(kt + 1) * d]
                        first, last = kt == 0, kt == KT - 1
                        for (lo, hi) in chunks_of(0, QBLK):
                            nc.tensor.matmul(ot_ps[:, lo:hi], vsl,
                                             pt[:, j * QBLK + lo:
                                                 j * QBLK + hi],
                                             start=first, stop=last)
                        # fp16 denominator accumulation on DVE (2x mode)
                        n = state["nadd"]
                        psl = pt[:, j * QBLK:(j + 1) * QBLK]
                        if n == 0:
                            nc.vector.tensor_copy(sacc[0][:], psl)
                        else:
                            nc.vector.tensor_add(sacc[n % 2][:],
                                                 sacc[(n + 1) % 2][:], psl)
                        state["nadd"] = n + 1

                st_unit(0)
                for u in range(npairs + 1):
                    if u + 1 <= npairs:
                        st_unit(u + 1)
                    av_unit(u)

                # ---- drain: denominators -> normalize -> transpose -> DMA ----
                sfin = sacc[(state["nadd"] + 1) % 2]
                sums = psum.tile([P, QBLK], f32, tag="sc", bufs=2,
                                 name=f"sums{pidx}")
                for (lo, hi) in chunks_of(0, QBLK):
                    nc.tensor.matmul(sums[:, lo:hi], onesm[:],
                                     sfin[:, lo:hi], start=True, stop=True)
                rinv = work.tile([P, QBLK], f32, tag="rinv", name=f"rinv{pidx}")
                nc.vector.reciprocal(rinv[:], sums[:, 0:QBLK])
                ot_sb = work.tile([P, QBLK], f32, tag="otsb", name=f"otsb{pidx}")
                nc.vector.tensor_mul(ot_sb[:], ot_ps[:, 0:QBLK], rinv[:])
                # transpose 6 chunks of 120 q rows; DMA straight from PSUM
                CH = QBLK // 6
                for j in range(6):
                    tp = psum.tile([CH, P], f32, tag="sc", bufs=2,
                                   name=f"tp{pidx}_{j}")
                    nc.tensor.transpose(tp[:, 0:P],
                                        ot_sb[:, j * CH:(j + 1) * CH],
                                        ident[:])
                    o_sb = work.tile([CH, P], f32, tag="osb", bufs=3,
                                     name=f"osb{pidx}_{j}")
                    nc.vector.tensor_copy(o_sb[:], tp[:, 0:P])
                    row0 = q0 + j * CH
                    nc.sync.dma_start(out_d[row0:row0 + CH, :], o_sb[:])

            run_pass(0, "a", passes[0])
            run_pass(1, "a", passes[1])
            run_pass(2, "b", passes[2])

    nc.finalize()
    meta = dict(QT_N=QT_N, n_kv_pad=n_kv_pad, knew=knew,
                last_valid=P - kpad)
    return nc, meta


# ----------------------------------------------------------------------------
# host wrapper
# ----------------------------------------------------------------------------

def kernel(q, k, v, k_cache, v_cache, current_start, global_end_index,
           local_end_index, grid_f, grid_h, grid_w):
    from concourse.bass_utils import run_bass_kernel_spmd

    q = np.asarray(q, dtype=np.float32)
    k = np.asarray(k, dtype=np.float32)
    v = np.asarray(v, dtype=np.float32)
    k_cache = np.asarray(k_cache, dtype=np.float32)
    v_cache = np.asarray(v_cache, dtype=np.float32)
    current_start = int(current_start)
    global_end_index = int(global_end_index)
    local_end_index = int(local_end_index)
    grid_h, grid_w = int(grid_h), int(grid_w)

    B, L, n_heads, d = q.shape
    cache_len = k_cache.shape[1]
    frame_seqlen = grid_h * grid_w
    start_frame = current_start // frame_seqlen

    segs, local_end, kv_start = _plan_cache_segments(
        current_start, global_end_index, local_end_index, L, cache_len,
        frame_seqlen)
    n_cache = sum(hi - lo for lo, hi in segs)
    n_kv = n_cache + L

    key = (L, d, n_cache, n_kv)
    if key not in _BUILD_CACHE:
        _BUILD_CACHE[key] = _build_program(L, d, n_cache, n_kv)
    nc, meta = _BUILD_CACHE[key]
    QT_N, n_kv_pad, knew = meta["QT_N"], meta["n_kv_pad"], meta["knew"]
    last_valid = meta["last_valid"]

    # gather the cache window once (numpy)
    kc_full = np.concatenate([k_cache[0, lo:hi] for lo, hi in segs], axis=0)
    vc_full = np.concatenate([v_cache[0, lo:hi] for lo, hi in segs], axis=0)

    cos_t, sin_t = _rope_cos_sin(L, d, grid_h, grid_w, start_frame)  # [L, 64]
    H = d // 2
    perm = np.concatenate([np.arange(0, d, 2), np.arange(1, d, 2)])

    biasm = np.zeros((P, 1), dtype=np.float32)
    biasm[last_valid:, 0] = -30.0
    ident = np.eye(P, dtype=np.float32)
    onesm = np.ones((P, P), dtype=np.float16)

    def dei_T(x):  # [rows, d] -> de-interleaved transpose [d, rows]
        return np.ascontiguousarray(x.T[perm])

    half = L // 2
    n_pairs = n_heads // 3
    assert n_heads % 3 == 0 and n_pairs * 2 == 8, "sharding expects 12 heads/8 cores"

    in_maps = []
    core_heads = []
    for c in range(8):
        p, s = c // 2, c % 2
        headA = 3 * p if s == 0 else 3 * p + 2
        headB = 3 * p + 1
        qsl = slice(0, half) if s == 0 else slice(half, L)
        core_heads.append((headA, headB, qsl))

        cosq = np.ones((P, QT_N), dtype=np.float32)
        sinq = np.zeros((P, QT_N), dtype=np.float32)
        for (c0, tab) in ((0, slice(0, L)), (L, qsl)):
            ct, st = cos_t[tab].T, sin_t[tab].T
            w = ct.shape[1]
            cosq[0:H, c0:c0 + w] = ct
            cosq[H:P, c0:c0 + w] = ct
            sinq[0:H, c0:c0 + w] = -st
            sinq[H:P, c0:c0 + w] = st

        qt = np.zeros((P, QT_N), dtype=np.float32)
        qt[:, 0:L] = dei_T(q[0, :, headA, :])
        qt[:, L:QT_N] = dei_T(q[0, qsl, headB, :])

        im = {"qt": qt, "cosq": cosq, "sinq": sinq,
              "biasm": biasm, "ident": ident, "ones": onesm}
        for tag, h in (("a", headA), ("b", headB)):
            ktn = np.zeros((P, knew), dtype=np.float32)
            ktn[:, 0:L] = dei_T(k[0, :, h, :])
            im[f"kt{tag}"] = ktn
            im[f"kc{tag}"] = dei_T(kc_full[:, h, :]).astype(np.float16)
            vaa = np.zeros((n_kv_pad, d), dtype=np.float16)
            vaa[0:n_cache] = vc_full[:, h, :]
            vaa[n_cache:n_cache + L] = v[0, :, h, :]
            im[f"va{tag}"] = vaa
        in_maps.append(im)

    res = run_bass_kernel_spmd(nc, in_maps, core_ids=list(range(8)))

    out = np.empty((B, L, n_heads, d), dtype=np.float32)
    for c in range(8):
        headA, headB, qsl = core_heads[c]
        o = res.results[c]["o"]
        out[0, :, headA, :] = o[0:L]
        out[0, qsl, headB, :] = o[L:L + half]
    return out


# revision 9
# speedup vs baseline: 1.2639x; 1.0258x over previous
"""Trainium2 Bass kernel for windowed (sink/ring-buffer) self-attention with RoPE.

Contract: kernel(**inputs) takes FULL unsharded inputs (as produced by the
problem's setup_inputs) and returns the FULL output [B, L, n, d].

Sharding: 12 heads x 1440 queries are split across 8 NeuronCores as
1.5 "head-units" per core: each core owns one full head (1440 queries) plus
half of a head shared with its pair core (720 queries). All cores run the
same SPMD program on differently-sliced inputs.

Device program (per core), v2 (fp16 datapath):
  - All matmul operands fp16 (q/k roped on-chip into fp16, v fp16 from host,
    exp output fp16). PSUM accumulation stays fp32.
  - Both KV head-slots live in SBUF simultaneously (fp16 halves the
    footprint), so all DMAs are issued up front and no pass stalls on a
    cache reload.
  - 3 uniform passes of 720 q columns over 85 kv tiles. Scores for two kv
    tiles are computed into one [128, 1440] PSUM tile so a single ScalarE
    activation (exp) covers both, halving per-instruction overhead on the
    bottleneck engine.
  - Softmax denominators: DVE accumulates exp tiles in fp16 (2x mode); the
    padded kv lanes of the last (ragged) tile are killed inside the exp
    itself via a per-partition bias of -30. The final partition-reduction is
    one ones-matmul on PE; output is normalized BEFORE the transpose so the
    transposed result DMAs straight from PSUM to DRAM.
"""

import math

import numpy as np

P = 128
THETA = 10000.0
LOCAL_ATTN_SIZE = 15
SINK_SIZE = 1

QBLK = 720          # q columns per pass (3 uniform passes over 2160 cols)

_BUILD_CACHE = {}


# ----------------------------------------------------------------------------
# host-side planning (mirrors the reference's python-int index logic)
# ----------------------------------------------------------------------------

def _plan_cache_segments(current_start, global_end_index, local_end_index,
                         num_new, cache_len, frame_seqlen):
    """Return (segments, local_end, kv_start): list of (lo, hi) slices of the
    ORIGINAL cache arrays that make up the pre-new-token part of the attention
    window, mirroring reference.py's roll/evict logic."""
    current_end = current_start + num_new
    sink_tokens = SINK_SIZE * frame_seqlen
    max_attn = LOCAL_ATTN_SIZE * frame_seqlen
    if current_end > global_end_index and num_new + local_end_index > cache_len:
        n_evict = num_new + local_end_index - cache_len
        n_roll = local_end_index - n_evict - sink_tokens
        local_end = local_end_index + current_end - global_end_index - n_evict
        roll_lo, roll_hi = sink_tokens, sink_tokens + n_roll

        def old_index(i):
            return i + n_evict if roll_lo <= i < roll_hi else i
    else:
        local_end = local_end_index + current_end - global_end_index
        n_evict = 0

        def old_index(i):
            return i

    local_start = local_end - num_new
    kv_start = max(0, local_end - max_attn)
    # contiguous segments of old_index over [kv_start, local_start)
    segs = []
    i = kv_start
    while i < local_start:
        lo = old_index(i)
        j = i
        while j + 1 < local_start and old_index(j + 1) == old_index(j) + 1:
            j += 1
        segs.append((lo, lo + (j - i + 1)))
        i = j + 1
    return segs, local_end, kv_start


def _rope_cos_sin(L, d, grid_h, grid_w, start_frame):
    """cos/sin angle tables [L, d//2] matching reference make_freqs/rope_apply."""
    c = d // 2
    d1 = d - 4 * (d // 6)
    d2 = 2 * (d // 6)
    inv1 = THETA ** (-(np.arange(0, d1, 2, dtype=np.float32) / np.float32(d1)))
    inv2 = THETA ** (-(np.arange(0, d2, 2, dtype=np.float32) / np.float32(d2)))
    inv3 = inv2
    hw = grid_h * grid_w
    pos = np.arange(L)
    f = pos // hw + start_frame
    hh = (pos % hw) // grid_w
    ww = pos % grid_w
    ang = np.concatenate([
        f[:, None].astype(np.float32) * inv1[None, :],
        hh[:, None].astype(np.float32) * inv2[None, :],
        ww[:, None].astype(np.float32) * inv3[None, :],
    ], axis=1)
    assert ang.shape == (L, c)
    return np.cos(ang).astype(np.float32), np.sin(ang).astype(np.float32)


# ----------------------------------------------------------------------------
# device program
# ----------------------------------------------------------------------------

def _build_program(L, d, n_cache, n_kv):
    """Build the SPMD Bass program for one core.

    L: new-token count (1440); d: head dim (128); n_cache: cache rows in the
    window (9360); n_kv: total kv rows (10800)."""
    import concourse.bass as bass
    import concourse.mybir as mybir
    import concourse.tile as tile
    from concourse import bacc

    f16 = mybir.dt.float16
    f32 = mybir.dt.float32
    Exp = mybir.ActivationFunctionType.Exp

    n_kv_pad = ((n_kv + P - 1) // P) * P          # 10880
    KT = n_kv_pad // P                            # 85 k-tiles
    npairs = KT // 2                              # 42 merged pairs
    kpad = n_kv_pad - n_kv                        # 80 zero-padded kv rows
    knew = n_kv_pad - n_cache                     # 1520 = new k cols + pad
    scale = 1.0 / math.sqrt(d)

    QT_N = 3 * QBLK                               # 2160 total q columns
    passes = (0, QBLK, 2 * QBLK)

    nc = bacc.Bacc(None, target_bir_lowering=False)

    qt_d = nc.dram_tensor("qt", [P, QT_N], f32, kind="ExternalInput")
    cosq_d = nc.dram_tensor("cosq", [P, QT_N], f32, kind="ExternalInput")
    sinq_d = nc.dram_tensor("sinq", [P, QT_N], f32, kind="ExternalInput")
    kt_d = {s: nc.dram_tensor(f"kt{s}", [P, knew], f32,
                              kind="ExternalInput") for s in "ab"}
    kc_d = {s: nc.dram_tensor(f"kc{s}", [P, n_cache], f16,
                              kind="ExternalInput") for s in "ab"}
    # va is host-pre-tiled to the on-chip layout [P, KT*d]:
    # va[p, t*d + j] = v[t*P + p, j] — DMA is then plain 2D contiguous
    va_d = {s: nc.dram_tensor(f"va{s}", [P, n_kv_pad], f16,
                              kind="ExternalInput") for s in "ab"}
    bias_d = nc.dram_tensor("biasm", [P, 1], f32, kind="ExternalInput")
    ident_d = nc.dram_tensor("ident", [P, P], f32, kind="ExternalInput")
    ones_d = nc.dram_tensor("ones", [P, P], f16, kind="ExternalInput")
    out_d = nc.dram_tensor("o", [QT_N, d], f32, kind="ExternalOutput")

    with tile.TileContext(nc) as tc:
        with tc.tile_pool(name="big", bufs=1) as big, \
             tc.tile_pool(name="work", bufs=2) as work, \
             tc.tile_pool(name="psum", bufs=1, space="PSUM") as psum:

            ident = big.tile([P, P], f32, tag="ident", name="ident")
            onesm = big.tile([P, P], f16, tag="onesm", name="onesm")
            biasm = big.tile([P, 1], f32, tag="biasm", name="biasm")

            cosq = big.tile([P, QT_N], f32, tag="cosq", name="cosq")
            sinq = big.tile([P, QT_N], f32, tag="sinq", name="sinq")

            rq = big.tile([P, QT_N], f16, tag="rq", name="rq")
            ka = {s: big.tile([P, n_kv_pad], f16, tag=f"ka{s}", name=f"ka{s}")
                  for s in "ab"}
            va = {s: big.tile([P, n_kv_pad], f16, tag=f"va{s}", name=f"va{s}")
                  for s in "ab"}

            def rope(dst_f16, src_f32, swap_f32, n_cols, tab_off, chunk):
                """dst = rope(src): y = src*C + swap*S with the sign folded
                into S; fp32 inputs, fp16 output, chunked so downstream
                matmuls unblock early."""
                for c0 in range(0, n_cols, chunk):
                    w = min(chunk, n_cols - c0)
                    C = cosq[:, tab_off + c0:tab_off + c0 + w]
                    S = sinq[:, tab_off + c0:tab_off + c0 + w]
                    t1 = work.tile([P, 768], f32, tag="ropet1", name="ropet1")
                    t2 = work.tile([P, 768], f32, tag="ropet2", name="ropet2")
                    nc.vector.tensor_mul(t1[:, 0:w], swap_f32[:, c0:c0 + w], S)
                    nc.vector.tensor_mul(t2[:, 0:w], src_f32[:, c0:c0 + w], C)
                    nc.vector.tensor_add(dst_f16[:, c0:c0 + w],
                                         t2[:, 0:w], t1[:, 0:w])

            # --- DMA issue order is tuned so the first score matmul and the
            # first exp unblock within a few microseconds: kc_a chunk 0 and
            # the first q/rope chunk go first, bulk data streams in behind ---
            H = P // 2
            qst = work.tile([P, QT_N], f32, tag="qst", bufs=1, name="qst")
            qsw = work.tile([P, QT_N], f32, tag="qsw", bufs=1, name="qsw")

            def dma_kc(s, lo, hi):
                nc.sync.dma_start(ka[s][:, lo:hi], kc_d[s][:, lo:hi])

            def dma_va(s, lo, hi):
                nc.sync.dma_start(va[s][:, lo:hi], va_d[s][:, lo:hi])

            def dma_qchunk(c0):
                c1 = c0 + QBLK
                nc.sync.dma_start(qst[:, c0:c1], qt_d[:, c0:c1])
                nc.sync.dma_start(qsw[0:H, c0:c1], qt_d[H:P, c0:c1])
                nc.sync.dma_start(qsw[H:P, c0:c1], qt_d[0:H, c0:c1])
                nc.sync.dma_start(cosq[:, c0:c1], cosq_d[:, c0:c1])
                nc.sync.dma_start(sinq[:, c0:c1], sinq_d[:, c0:c1])

            kst = {}
            ksw = {}

            def dma_kv_slot(s):
                kst[s] = work.tile([P, knew], f32, tag=f"kst{s}", bufs=1,
                                   name=f"kst{s}")
                ksw[s] = work.tile([P, knew], f32, tag=f"ksw{s}", bufs=1,
                                   name=f"ksw{s}")
                nc.sync.dma_start(kst[s][:], kt_d[s][:])
                nc.sync.dma_start(ksw[s][0:H, :], kt_d[s][H:P, :])
                nc.sync.dma_start(ksw[s][H:P, :], kt_d[s][0:H, :])

            def rope_k_slot(s):
                rope(ka[s][:, n_cache:n_kv_pad], kst[s], ksw[s], knew, 0,
                     knew // 2)

            KCCH = n_cache // 8
            VACH = n_kv_pad // 8
            dma_kc("a", 0, KCCH)
            dma_qchunk(0)
            dma_va("a", 0, VACH)
            for cidx in range(1, 8):
                dma_kc("a", cidx * KCCH, n_cache if cidx == 7
                       else (cidx + 1) * KCCH)
            for cidx in range(1, 8):
                dma_va("a", cidx * VACH, (cidx + 1) * VACH)
            dma_qchunk(QBLK)
            dma_qchunk(2 * QBLK)
            dma_kv_slot("a")
            for cidx in range(8):
                dma_kc("b", cidx * KCCH, n_cache if cidx == 7
                       else (cidx + 1) * KCCH)
            for cidx in range(8):
                dma_va("b", cidx * VACH, (cidx + 1) * VACH)
            dma_kv_slot("b")
            nc.sync.dma_start(ident[:], ident_d[:])
            nc.sync.dma_start(onesm[:], ones_d[:])
            nc.sync.dma_start(biasm[:], bias_d[:])

            rope(rq, qst, qsw, QT_N, 0, QBLK)
            rope_k_slot("a")

            # fp16 softmax-denominator accumulators: a DVE ping-pong chain
            # plus a GpSimd ping-pong chain (every 4th tile) to offload DVE
            sacc = [big.tile([P, QBLK], f16, tag=f"sacc{i}", name=f"sacc{i}")
                    for i in range(2)]
            sagp = [big.tile([P, QBLK], f16, tag=f"sagp{i}", name=f"sagp{i}")
                    for i in range(2)]

            def chunks_of(lo, hi):
                """Split [lo, hi) at 512-col PSUM bank boundaries."""
                cuts = [lo]
                b = (lo // 512 + 1) * 512
                while b < hi:
                    cuts.append(b)
                    b += 512
                cuts.append(hi)
                return list(zip(cuts[:-1], cuts[1:]))

            CH = QBLK // 6
            # drain PSUM region layout (one 3-bank sc-pool slot):
            # [0:720) sums, [762:768) transposed denominator columns,
            # [768:1536) six transposed 120x128 output chunks
            DR_STD = 762
            DR_OTT = 768

            def run_pass(pidx, slot, q0, drain_prev):
                """One 720-wide q pass over 85 kv tiles (42 pairs + 1 single).
                Software-pipelined one pair deep; the previous pass's drain is
                issued after this pass's first two score units so its exp
                stream never starves."""
                kas, vas = ka[slot], va[slot]
                ot_ps = psum.tile([P, QBLK], f32, tag="ot", name=f"ot{pidx}")

                pts = {}
                state = dict(nadd=0, ndve=0, ngp=0)

                def st_unit(u):
                    """Scores for unit u into one PSUM tile + one exp."""
                    single = u == npairs
                    w = QBLK if single else 2 * QBLK
                    sc = psum.tile([P, 2 * QBLK], f32, tag="sc", bufs=2,
                                   name=f"sc{pidx}_{u}")
                    for j in range(1 if single else 2):
                        kt = 2 * u + j
                        ksl = kas[:, kt * P:(kt + 1) * P]
                        for (lo, hi) in chunks_of(j * QBLK, (j + 1) * QBLK):
                            nc.tensor.matmul(sc[:, lo:hi], ksl,
                                             rq[:, q0 + lo - j * QBLK:
                                                 q0 + hi - j * QBLK],
                                             start=True, stop=True)
                    pt = work.tile([P, 2 * QBLK], f16, tag="pt", bufs=6,
                                   name=f"pt{pidx}_{u}")
                    if single:
                        # bias kills the zero-padded kv lanes (exp ~ 0)
                        nc.scalar.activation(pt[:, 0:w], sc[:, 0:w], Exp,
                                             bias=biasm[:, 0:1], scale=scale)
                    else:
                        nc.scalar.activation(pt[:, 0:w], sc[:, 0:w], Exp,
                                             scale=scale)
                    pts[u] = pt

                def av_unit(u):
                    pt = pts.pop(u)
                    single = u == npairs
                    for j in range(1 if single else 2):
                        kt = 2 * u + j
                        vsl = vas[:, kt * d:(kt + 1) * d]
                        first, last = kt == 0, kt == KT - 1
                        for (lo, hi) in chunks_of(0, QBLK):
                            nc.tensor.matmul(ot_ps[:, lo:hi], vsl,
                                             pt[:, j * QBLK + lo:
                                                 j * QBLK + hi],
                                             start=first, stop=last)
                        # fp16 denominator accumulation (DVE 2x / GpSimd)
                        n = state["nadd"]
                        psl = pt[:, j * QBLK:(j + 1) * QBLK]
                        if n % 4 == 2:
                            g = state["ngp"]
                            if g == 0:
                                nc.gpsimd.tensor_copy(sagp[0][:], psl)
                            else:
                                nc.gpsimd.tensor_add(sagp[g % 2][:],
                                                     sagp[(g + 1) % 2][:],
                                                     psl)
                            state["ngp"] = g + 1
                        else:
                            v_ = state["ndve"]
                            if v_ == 0:
                                nc.vector.tensor_copy(sacc[0][:], psl)
                            else:
                                nc.vector.tensor_add(sacc[v_ % 2][:],
                                                     sacc[(v_ + 1) % 2][:],
                                                     psl)
                            state["ndve"] = v_ + 1
                        state["nadd"] = n + 1

                st_unit(0)
                st_unit(1)
                if drain_prev is not None:
                    drain_prev()
                for u in range(npairs + 1):
                    if u + 2 <= npairs:
                        st_unit(u + 2)
                    av_unit(u)

                def drain():
                    """Denominator fold + normalize + transpose + store for
                    this finished pass (issued from inside the next pass)."""
                    sfin = sacc[(state["ndve"] + 1) % 2]
                    gfin = sagp[(state["ngp"] + 1) % 2]
                    ot_sb = work.tile([P, QBLK], f32, tag="otsb",
                                      name=f"otsb{pidx}")
                    nc.vector.tensor_copy(ot_sb[:], ot_ps[:, 0:QBLK])
                    dr = psum.tile([P, 3 * 512], f32, tag="sc", bufs=2,
                                   name=f"dr{pidx}")
                    for (lo, hi) in chunks_of(0, QBLK):
                        nc.tensor.matmul(dr[:, lo:hi], onesm[:],
                                         sfin[:, lo:hi],
                                         start=True, stop=False)
                        nc.tensor.matmul(dr[:, lo:hi], onesm[:],
                                         gfin[:, lo:hi],
                                         start=False, stop=True)
                    s_sb = work.tile([1, QBLK], f32, tag="ssb", bufs=1,
                                     name=f"ssb{pidx}")
                    nc.vector.tensor_copy(s_sb[0:1, :], dr[0:1, 0:QBLK])
                    for j in range(6):
                        nc.tensor.transpose(dr[0:CH, DR_STD + j:DR_STD + j + 1],
                                            s_sb[0:1, j * CH:(j + 1) * CH],
                                            ident[0:1, 0:1])
                        nc.tensor.transpose(dr[0:CH, DR_OTT + j * P:
                                               DR_OTT + (j + 1) * P],
                                            ot_sb[:, j * CH:(j + 1) * CH],
                                            ident[:])
                    for j in range(6):
                        r_sb = work.tile([CH, 1], f32, tag="rsb", bufs=2,
                                         name=f"rsb{pidx}_{j}")
                        nc.vector.reciprocal(
                            r_sb[:], dr[0:CH, DR_STD + j:DR_STD + j + 1])
                        o_sb = work.tile([CH, P], f32, tag="osb", bufs=3,
                                         name=f"osb{pidx}_{j}")
                        nc.vector.tensor_scalar_mul(
                            o_sb[:], dr[0:CH, DR_OTT + j * P:
                                         DR_OTT + (j + 1) * P], r_sb[:])
                        row0 = q0 + j * CH
                        nc.sync.dma_start(out_d[row0:row0 + CH, :], o_sb[:])

                return drain

            d0 = run_pass(0, "a", passes[0], None)
            rope_k_slot("b")
            d1 = run_pass(1, "a", passes[1], d0)
            d2 = run_pass(2, "b", passes[2], d1)
            d2()

    nc.finalize()
    meta = dict(QT_N=QT_N, n_kv_pad=n_kv_pad, knew=knew,
                last_valid=P - kpad)
    return nc, meta


# ----------------------------------------------------------------------------
# host wrapper
# ----------------------------------------------------------------------------

def kernel(q, k, v, k_cache, v_cache, current_start, global_end_index,
           local_end_index, grid_f, grid_h, grid_w):
    from concourse.bass_utils import run_bass_kernel_spmd

    q = np.asarray(q, dtype=np.float32)
    k = np.asarray(k, dtype=np.float32)
    v = np.asarray(v, dtype=np.float32)
    k_cache = np.asarray(k_cache, dtype=np.float32)
    v_cache = np.asarray(v_cache, dtype=np.float32)
    current_start = int(current_start)
    global_end_index = int(global_end_index)
    local_end_index = int(local_end_index)
    grid_h, grid_w = int(grid_h), int(grid_w)

    B, L, n_heads, d = q.shape
    cache_len = k_cache.shape[1]
    frame_seqlen = grid_h * grid_w
    start_frame = current_start // frame_seqlen

    segs, local_end, kv_start = _plan_cache_segments(
        current_start, global_end_index, local_end_index, L, cache_len,
        frame_seqlen)
    n_cache = sum(hi - lo for lo, hi in segs)
    n_kv = n_cache + L

    key = (L, d, n_cache, n_kv)
    if key not in _BUILD_CACHE:
        _BUILD_CACHE[key] = _build_program(L, d, n_cache, n_kv)
    nc, meta = _BUILD_CACHE[key]
    QT_N, n_kv_pad, knew = meta["QT_N"], meta["n_kv_pad"], meta["knew"]
    last_valid = meta["last_valid"]

    # gather the cache window once (numpy)
    kc_full = np.concatenate([k_cache[0, lo:hi] for lo, hi in segs], axis=0)
    vc_full = np.concatenate([v_cache[0, lo:hi] for lo, hi in segs], axis=0)

    cos_t, sin_t = _rope_cos_sin(L, d, grid_h, grid_w, start_frame)  # [L, 64]
    H = d // 2
    perm = np.concatenate([np.arange(0, d, 2), np.arange(1, d, 2)])

    biasm = np.zeros((P, 1), dtype=np.float32)
    biasm[last_valid:, 0] = -30.0
    ident = np.eye(P, dtype=np.float32)
    onesm = np.ones((P, P), dtype=np.float16)

    def dei_T(x):  # [rows, d] -> de-interleaved transpose [d, rows]
        return np.ascontiguousarray(x.T[perm])

    half = L // 2
    n_pairs = n_heads // 3
    assert n_heads % 3 == 0 and n_pairs * 2 == 8, "sharding expects 12 heads/8 cores"

    in_maps = []
    core_heads = []
    for c in range(8):
        p, s = c // 2, c % 2
        headA = 3 * p if s == 0 else 3 * p + 2
        headB = 3 * p + 1
        qsl = slice(0, half) if s == 0 else slice(half, L)
        core_heads.append((headA, headB, qsl))

        cosq = np.ones((P, QT_N), dtype=np.float32)
        sinq = np.zeros((P, QT_N), dtype=np.float32)
        for (c0, tab) in ((0, slice(0, L)), (L, qsl)):
            ct, st = cos_t[tab].T, sin_t[tab].T
            w = ct.shape[1]
            cosq[0:H, c0:c0 + w] = ct
            cosq[H:P, c0:c0 + w] = ct
            sinq[0:H, c0:c0 + w] = -st
            sinq[H:P, c0:c0 + w] = st

        qt = np.zeros((P, QT_N), dtype=np.float32)
        qt[:, 0:L] = dei_T(q[0, :, headA, :])
        qt[:, L:QT_N] = dei_T(q[0, qsl, headB, :])

        im = {"qt": qt, "cosq": cosq, "sinq": sinq,
              "biasm": biasm, "ident": ident, "ones": onesm}
        for tag, h in (("a", headA), ("b", headB)):
            ktn = np.zeros((P, knew), dtype=np.float32)
            ktn[:, 0:L] = dei_T(k[0, :, h, :])
            im[f"kt{tag}"] = ktn
            im[f"kc{tag}"] = dei_T(kc_full[:, h, :]).astype(np.float16)
            vaa = np.zeros((n_kv_pad, d), dtype=np.float16)
            vaa[0:n_cache] = vc_full[:, h, :]
            vaa[n_cache:n_cache + L] = v[0, :, h, :]
            # pre-tile to the on-chip layout [P, KT*d]
            im[f"va{tag}"] = np.ascontiguousarray(
                vaa.reshape(n_kv_pad // P, P, d).transpose(1, 0, 2)
                .reshape(P, n_kv_pad))
        in_maps.append(im)

    res = run_bass_kernel_spmd(nc, in_maps, core_ids=list(range(8)))

    out = np.empty((B, L, n_heads, d), dtype=np.float32)
    for c in range(8):
        headA, headB, qsl = core_heads[c]
        o = res.results[c]["o"]
        out[0, :, headA, :] = o[0:L]
        out[0, qsl, headB, :] = o[L:L + half]
    return out


# revision 13
# speedup vs baseline: 1.5179x; 1.2010x over previous
"""Trainium2 Bass kernel for windowed (sink/ring-buffer) self-attention with RoPE.

Contract: kernel(**inputs) takes FULL unsharded inputs (as produced by the
problem's setup_inputs) and returns the FULL output [B, L, n, d].

Sharding: 12 heads x 1440 queries are split across 8 NeuronCores as
1.5 "head-units" per core: each core owns one full head (1440 queries) plus
half of a head shared with its pair core (720 queries). All cores run the
same SPMD program on differently-sliced inputs.

Device program (per core), v2 (fp16 datapath):
  - All matmul operands fp16 (q/k roped on-chip into fp16, v fp16 from host,
    exp output fp16). PSUM accumulation stays fp32.
  - Both KV head-slots live in SBUF simultaneously (fp16 halves the
    footprint), so all DMAs are issued up front and no pass stalls on a
    cache reload.
  - 3 uniform passes of 720 q columns over 85 kv tiles. Scores for two kv
    tiles are computed into one [128, 1440] PSUM tile so a single ScalarE
    activation (exp) covers both, halving per-instruction overhead on the
    bottleneck engine.
  - Softmax denominators: DVE accumulates exp tiles in fp16 (2x mode); the
    padded kv lanes of the last (ragged) tile are killed inside the exp
    itself via a per-partition bias of -30. The final partition-reduction is
    one ones-matmul on PE; output is normalized BEFORE the transpose so the
    transposed result DMAs straight from PSUM to DRAM.
"""

import math

import numpy as np

P = 128
THETA = 10000.0
LOCAL_ATTN_SIZE = 15
SINK_SIZE = 1

QBLK = 720          # q columns per pass (3 uniform passes over 2160 cols)

_BUILD_CACHE = {}


# ----------------------------------------------------------------------------
# host-side planning (mirrors the reference's python-int index logic)
# ----------------------------------------------------------------------------

def _plan_cache_segments(current_start, global_end_index, local_end_index,
                         num_new, cache_len, frame_seqlen):
    """Return (segments, local_end, kv_start): list of (lo, hi) slices of the
    ORIGINAL cache arrays that make up the pre-new-token part of the attention
    window, mirroring reference.py's roll/evict logic."""
    current_end = current_start + num_new
    sink_tokens = SINK_SIZE * frame_seqlen
    max_attn = LOCAL_ATTN_SIZE * frame_seqlen
    if current_end > global_end_index and num_new + local_end_index > cache_len:
        n_evict = num_new + local_end_index - cache_len
        n_roll = local_end_index - n_evict - sink_tokens
        local_end = local_end_index + current_end - global_end_index - n_evict
        roll_lo, roll_hi = sink_tokens, sink_tokens + n_roll

        def old_index(i):
            return i + n_evict if roll_lo <= i < roll_hi else i
    else:
        local_end = local_end_index + current_end - global_end_index
        n_evict = 0

        def old_index(i):
            return i

    local_start = local_end - num_new
    kv_start = max(0, local_end - max_attn)
    # contiguous segments of old_index over [kv_start, local_start)
    segs = []
    i = kv_start
    while i < local_start:
        lo = old_index(i)
        j = i
        while j + 1 < local_start and old_index(j + 1) == old_index(j) + 1:
            j += 1
        segs.append((lo, lo + (j - i + 1)))
        i = j + 1
    return segs, local_end, kv_start


def _rope_cos_sin(L, d, grid_h, grid_w, start_frame):
    """cos/sin angle tables [L, d//2] matching reference make_freqs/rope_apply."""
    c = d // 2
    d1 = d - 4 * (d // 6)
    d2 = 2 * (d // 6)
    inv1 = THETA ** (-(np.arange(0, d1, 2, dtype=np.float32) / np.float32(d1)))
    inv2 = THETA ** (-(np.arange(0, d2, 2, dtype=np.float32) / np.float32(d2)))
    inv3 = inv2
    hw = grid_h * grid_w
    pos = np.arange(L)
    f = pos // hw + start_frame
    hh = (pos % hw) // grid_w
    ww = pos % grid_w
    ang = np.concatenate([
        f[:, None].astype(np.float32) * inv1[None, :],
        hh[:, None].astype(np.float32) * inv2[None, :],
        ww[:, None].astype(np.float32) * inv3[None, :],
    ], axis=1)
    assert ang.shape == (L, c)
    return np.cos(ang).astype(np.float32), np.sin(ang).astype(np.float32)


# ----------------------------------------------------------------------------
# device program
# ----------------------------------------------------------------------------

def _build_program(L, d, n_cache, n_kv):
    """Build the SPMD Bass program for one core.

    L: new-token count (1440); d: head dim (128); n_cache: cache rows in the
    window (9360); n_kv: total kv rows (10800)."""
    import concourse.bass as bass
    import concourse.mybir as mybir
    import concourse.tile as tile
    from concourse import bacc

    f16 = mybir.dt.float16
    f32 = mybir.dt.float32
    Exp = mybir.ActivationFunctionType.Exp

    n_kv_pad = ((n_kv + P - 1) // P) * P          # 10880
    KT = n_kv_pad // P                            # 85 k-tiles
    npairs = KT // 2                              # 42 merged pairs
    kpad = n_kv_pad - n_kv                        # 80 zero-padded kv rows
    knew = n_kv_pad - n_cache                     # 1520 = new k cols + pad
    scale = 1.0 / math.sqrt(d)

    QT_N = 3 * QBLK                               # 2160 total q columns
    passes = (0, QBLK, 2 * QBLK)

    nc = bacc.Bacc(None, target_bir_lowering=False)

    qt_d = nc.dram_tensor("qt", [P, QT_N], f32, kind="ExternalInput")
    cosq_d = nc.dram_tensor("cosq", [P, QT_N], f32, kind="ExternalInput")
    sinq_d = nc.dram_tensor("sinq", [P, QT_N], f32, kind="ExternalInput")
    kt_d = {s: nc.dram_tensor(f"kt{s}", [P, knew], f32,
                              kind="ExternalInput") for s in "ab"}
    kc_d = {s: nc.dram_tensor(f"kc{s}", [P, n_cache], f16,
                              kind="ExternalInput") for s in "ab"}
    # va is host-pre-tiled to the on-chip layout [P, KT*d]:
    # va[p, t*d + j] = v[t*P + p, j] — DMA is then plain 2D contiguous
    va_d = {s: nc.dram_tensor(f"va{s}", [P, n_kv_pad], f16,
                              kind="ExternalInput") for s in "ab"}
    bias_d = nc.dram_tensor("biasm", [P, 1], f32, kind="ExternalInput")
    ident_d = nc.dram_tensor("ident", [P, P], f32, kind="ExternalInput")
    ones_d = nc.dram_tensor("ones", [P, P], f16, kind="ExternalInput")
    out_d = nc.dram_tensor("o", [QT_N, d], f32, kind="ExternalOutput")

    with tile.TileContext(nc) as tc:
        with tc.tile_pool(name="big", bufs=1) as big, \
             tc.tile_pool(name="work", bufs=2) as work, \
             tc.tile_pool(name="psum", bufs=1, space="PSUM") as psum:

            ident = big.tile([P, P], f32, tag="ident", name="ident")
            onesm = big.tile([P, P], f16, tag="onesm", name="onesm")
            biasm = big.tile([P, 1], f32, tag="biasm", name="biasm")

            cosq = big.tile([P, QT_N], f32, tag="cosq", name="cosq")
            sinq = big.tile([P, QT_N], f32, tag="sinq", name="sinq")

            rq = big.tile([P, QT_N], f16, tag="rq", name="rq")
            ka = {s: big.tile([P, n_kv_pad], f16, tag=f"ka{s}", name=f"ka{s}")
                  for s in "ab"}
            va = {s: big.tile([P, n_kv_pad], f16, tag=f"va{s}", name=f"va{s}")
                  for s in "ab"}

            def rope(dst_f16, src_f32, swap_f32, n_cols, tab_off, chunk):
                """dst = rope(src): y = src*C + swap*S with the sign folded
                into S; fp32 inputs, fp16 output, chunked so downstream
                matmuls unblock early."""
                for c0 in range(0, n_cols, chunk):
                    w = min(chunk, n_cols - c0)
                    C = cosq[:, tab_off + c0:tab_off + c0 + w]
                    S = sinq[:, tab_off + c0:tab_off + c0 + w]
                    t1 = work.tile([P, 768], f32, tag="ropet1", name="ropet1")
                    t2 = work.tile([P, 768], f32, tag="ropet2", name="ropet2")
                    nc.vector.tensor_mul(t1[:, 0:w], swap_f32[:, c0:c0 + w], S)
                    nc.vector.tensor_mul(t2[:, 0:w], src_f32[:, c0:c0 + w], C)
                    nc.vector.tensor_add(dst_f16[:, c0:c0 + w],
                                         t2[:, 0:w], t1[:, 0:w])

            # --- DMA issue order is tuned so the first score matmul and the
            # first exp unblock within a few microseconds: kc_a chunk 0 and
            # the first q/rope chunk go first, bulk data streams in behind ---
            H = P // 2
            qst = work.tile([P, QT_N], f32, tag="qst", bufs=1, name="qst")
            qsw = work.tile([P, QT_N], f32, tag="qsw", bufs=1, name="qsw")

            def dma_kc(s, lo, hi):
                nc.sync.dma_start(ka[s][:, lo:hi], kc_d[s][:, lo:hi])

            def dma_va(s, lo, hi):
                nc.sync.dma_start(va[s][:, lo:hi], va_d[s][:, lo:hi])

            def dma_qchunk(c0):
                c1 = c0 + QBLK
                nc.sync.dma_start(qst[:, c0:c1], qt_d[:, c0:c1])
                nc.sync.dma_start(qsw[0:H, c0:c1], qt_d[H:P, c0:c1])
                nc.sync.dma_start(qsw[H:P, c0:c1], qt_d[0:H, c0:c1])
                nc.sync.dma_start(cosq[:, c0:c1], cosq_d[:, c0:c1])
                nc.sync.dma_start(sinq[:, c0:c1], sinq_d[:, c0:c1])

            kst = {}
            ksw = {}

            def dma_kv_slot(s):
                kst[s] = work.tile([P, knew], f32, tag=f"kst{s}", bufs=1,
                                   name=f"kst{s}")
                ksw[s] = work.tile([P, knew], f32, tag=f"ksw{s}", bufs=1,
                                   name=f"ksw{s}")
                nc.sync.dma_start(kst[s][:], kt_d[s][:])
                nc.sync.dma_start(ksw[s][0:H, :], kt_d[s][H:P, :])
                nc.sync.dma_start(ksw[s][H:P, :], kt_d[s][0:H, :])

            def rope_k_slot(s):
                rope(ka[s][:, n_cache:n_kv_pad], kst[s], ksw[s], knew, 0,
                     knew // 2)

            KCCH = n_cache // 8
            VACH = n_kv_pad // 8
            dma_qchunk(0)
            dma_kc("a", 0, KCCH)
            dma_va("a", 0, VACH)
            for cidx in range(1, 8):
                dma_kc("a", cidx * KCCH, n_cache if cidx == 7
                       else (cidx + 1) * KCCH)
            for cidx in range(1, 8):
                dma_va("a", cidx * VACH, (cidx + 1) * VACH)
            dma_qchunk(QBLK)
            dma_qchunk(2 * QBLK)
            dma_kv_slot("a")
            for cidx in range(8):
                dma_kc("b", cidx * KCCH, n_cache if cidx == 7
                       else (cidx + 1) * KCCH)
            for cidx in range(8):
                dma_va("b", cidx * VACH, (cidx + 1) * VACH)
            dma_kv_slot("b")
            nc.sync.dma_start(ident[:], ident_d[:])
            nc.sync.dma_start(onesm[:], ones_d[:])
            nc.sync.dma_start(biasm[:], bias_d[:])

            rope(rq, qst, qsw, QT_N, 0, QBLK)
            rope_k_slot("a")

            # fp16 softmax-denominator accumulator: DVE ping-pong chain of
            # full-pair-width adds (one [P, 1440] add per exp unit)
            sacc = [big.tile([P, 2 * QBLK], f16, tag=f"sacc{i}",
                             name=f"sacc{i}") for i in range(2)]

            def chunks_of(lo, hi):
                """Split [lo, hi) at 512-col PSUM bank boundaries."""
                cuts = [lo]
                b = (lo // 512 + 1) * 512
                while b < hi:
                    cuts.append(b)
                    b += 512
                cuts.append(hi)
                return list(zip(cuts[:-1], cuts[1:]))

            CH = QBLK // 6
            # drain PSUM region layout (one 3-bank sc-pool slot):
            # [0:720) sums, [762:768) transposed denominator columns,
            # [768:1536) six transposed 120x128 output chunks
            DR_STD = 762
            DR_OTT = 768

            def run_pass(pidx, slot, q0, drain_head, drain_rest):
                """One 720-wide q pass over 85 kv tiles (42 pairs + 1 single).
                Software-pipelined one pair deep; the previous pass's drain is
                issued in two pieces (output copy up front on the then-idle
                ScalarE, the rest after four score units) so the exp stream
                never starves."""
                kas, vas = ka[slot], va[slot]
                ot_ps = psum.tile([P, QBLK], f32, tag="ot", name=f"ot{pidx}")

                pts = {}
                state = dict(nadd=0)

                def st_unit(u):
                    """Scores for unit u into one PSUM tile + one exp."""
                    single = u == npairs
                    w = QBLK if single else 2 * QBLK
                    sc = psum.tile([P, 2 * QBLK], f32, tag="sc", bufs=2,
                                   name=f"sc{pidx}_{u}")
                    for j in range(1 if single else 2):
                        kt = 2 * u + j
                        ksl = kas[:, kt * P:(kt + 1) * P]
                        for (lo, hi) in chunks_of(j * QBLK, (j + 1) * QBLK):
                            nc.tensor.matmul(sc[:, lo:hi], ksl,
                                             rq[:, q0 + lo - j * QBLK:
                                                 q0 + hi - j * QBLK],
                                             start=True, stop=True)
                    pt = work.tile([P, 2 * QBLK], f16, tag="pt", bufs=6,
                                   name=f"pt{pidx}_{u}")
                    if single:
                        # bias kills the zero-padded kv lanes (exp ~ 0)
                        nc.scalar.activation(pt[:, 0:w], sc[:, 0:w], Exp,
                                             bias=biasm[:, 0:1], scale=scale)
                    else:
                        nc.scalar.activation(pt[:, 0:w], sc[:, 0:w], Exp,
                                             scale=scale)
                    pts[u] = pt

                def av_unit(u):
                    pt = pts.pop(u)
                    single = u == npairs
                    for j in range(1 if single else 2):
                        kt = 2 * u + j
                        vsl = vas[:, kt * d:(kt + 1) * d]
                        first, last = kt == 0, kt == KT - 1
                        for (lo, hi) in chunks_of(0, QBLK):
                            nc.tensor.matmul(ot_ps[:, lo:hi], vsl,
                                             pt[:, j * QBLK + lo:
                                                 j * QBLK + hi],
                                             start=first, stop=last)
                    if single:
                        # the ragged tile skips the DVE chain; its
                        # denominator goes through two extra fold matmuls
                        state["pt84"] = pt
                    else:
                        # one full-pair-width fp16 add on DVE (2x mode)
                        n = state["nadd"]
                        if n == 0:
                            nc.vector.tensor_copy(sacc[0][:], pt[:, :])
                        else:
                            nc.vector.tensor_add(sacc[n % 2][:],
                                                 sacc[(n + 1) % 2][:],
                                                 pt[:, :])
                        state["nadd"] = n + 1

                st_unit(0)
                st_unit(1)
                if drain_head is not None:
                    drain_head()
                st_unit(2)
                st_unit(3)
                if drain_rest is not None:
                    drain_rest()
                for u in range(npairs + 1):
                    if u + 4 <= npairs:
                        st_unit(u + 4)
                    av_unit(u)

                ot_sb = work.tile([P, QBLK], f32, tag="otsb",
                                  name=f"otsb{pidx}")

                def head():
                    """Output copy on ScalarE, issued while it is idle at the
                    start of the next pass."""
                    nc.scalar.copy(ot_sb[:], ot_ps[:, 0:QBLK])

                def rest():
                    """Denominator fold + normalize + transpose + store."""
                    sfin = sacc[(state["nadd"] + 1) % 2]
                    pt84 = state["pt84"]
                    dr = psum.tile([P, 3 * 512], f32, tag="sc", bufs=2,
                                   name=f"dr{pidx}")
                    # fold both halves of the pair accumulator + the ragged
                    # tile's exp directly (kv-partition reduce via ones)
                    nc.tensor.matmul(dr[:, 0:512], onesm[:],
                                     sfin[:, 0:512], start=True, stop=False)
                    nc.tensor.matmul(dr[:, 0:512], onesm[:],
                                     sfin[:, QBLK:QBLK + 512],
                                     start=False, stop=False)
                    nc.tensor.matmul(dr[:, 0:512], onesm[:],
                                     pt84[:, 0:512], start=False, stop=True)
                    nc.tensor.matmul(dr[:, 512:QBLK], onesm[:],
                                     sfin[:, 512:QBLK], start=True, stop=False)
                    nc.tensor.matmul(dr[:, 512:QBLK], onesm[:],
                                     sfin[:, QBLK + 512:2 * QBLK],
                                     start=False, stop=False)
                    nc.tensor.matmul(dr[:, 512:QBLK], onesm[:],
                                     pt84[:, 512:QBLK],
                                     start=False, stop=True)
                    s_sb = work.tile([1, QBLK], f32, tag="ssb", bufs=1,
                                     name=f"ssb{pidx}")
                    nc.vector.tensor_copy(s_sb[0:1, :], dr[0:1, 0:QBLK])
                    for j in range(6):
                        nc.tensor.transpose(dr[0:CH, DR_STD + j:DR_STD + j + 1],
                                            s_sb[0:1, j * CH:(j + 1) * CH],
                                            ident[0:1, 0:1])
                        nc.tensor.transpose(dr[0:CH, DR_OTT + j * P:
                                               DR_OTT + (j + 1) * P],
                                            ot_sb[:, j * CH:(j + 1) * CH],
                                            ident[:])
                        r_sb = work.tile([CH, 1], f32, tag="rsb", bufs=2,
                                         name=f"rsb{pidx}_{j}")
                        nc.vector.reciprocal(
                            r_sb[:], dr[0:CH, DR_STD + j:DR_STD + j + 1])
                        o_sb = work.tile([CH, P], f32, tag="osb", bufs=3,
                                         name=f"osb{pidx}_{j}")
                        nc.vector.tensor_scalar_mul(
                            o_sb[:], dr[0:CH, DR_OTT + j * P:
                                         DR_OTT + (j + 1) * P], r_sb[:])
                        row0 = q0 + j * CH
                        nc.sync.dma_start(out_d[row0:row0 + CH, :], o_sb[:])

                return head, rest

            h0, r0 = run_pass(0, "a", passes[0], None, None)
            rope_k_slot("b")
            h1, r1 = run_pass(1, "a", passes[1], h0, r0)
            h2, r2 = run_pass(2, "b", passes[2], h1, r1)
            h2()
            r2()

    nc.finalize()
    meta = dict(QT_N=QT_N, n_kv_pad=n_kv_pad, knew=knew,
                last_valid=P - kpad)
    return nc, meta


# ----------------------------------------------------------------------------
# host wrapper
# ----------------------------------------------------------------------------

def kernel(q, k, v, k_cache, v_cache, current_start, global_end_index,
           local_end_index, grid_f, grid_h, grid_w):
    from concourse.bass_utils import run_bass_kernel_spmd

    q = np.asarray(q, dtype=np.float32)
    k = np.asarray(k, dtype=np.float32)
    v = np.asarray(v, dtype=np.float32)
    k_cache = np.asarray(k_cache, dtype=np.float32)
    v_cache = np.asarray(v_cache, dtype=np.float32)
    current_start = int(current_start)
    global_end_index = int(global_end_index)
    local_end_index = int(local_end_index)
    grid_h, grid_w = int(grid_h), int(grid_w)

    B, L, n_heads, d = q.shape
    cache_len = k_cache.shape[1]
    frame_seqlen = grid_h * grid_w
    start_frame = current_start // frame_seqlen

    segs, local_end, kv_start = _plan_cache_segments(
        current_start, global_end_index, local_end_index, L, cache_len,
        frame_seqlen)
    n_cache = sum(hi - lo for lo, hi in segs)
    n_kv = n_cache + L

    key = (L, d, n_cache, n_kv)
    if key not in _BUILD_CACHE:
        _BUILD_CACHE[key] = _build_program(L, d, n_cache, n_kv)
    nc, meta = _BUILD_CACHE[key]
    QT_N, n_kv_pad, knew = meta["QT_N"], meta["n_kv_pad"], meta["knew"]
    last_valid = meta["last_valid"]

    # gather the cache window once (numpy)
    kc_full = np.concatenate([k_cache[0, lo:hi] for lo, hi in segs], axis=0)
    vc_full = np.concatenate([v_cache[0, lo:hi] for lo, hi in segs], axis=0)

    cos_t, sin_t = _rope_cos_sin(L, d, grid_h, grid_w, start_frame)  # [L, 64]
    H = d // 2
    perm = np.concatenate([np.arange(0, d, 2), np.arange(1, d, 2)])

    biasm = np.zeros((P, 1), dtype=np.float32)
    biasm[last_valid:, 0] = -30.0
    ident = np.eye(P, dtype=np.float32)
    onesm = np.ones((P, P), dtype=np.float16)

    def dei_T(x):  # [rows, d] -> de-interleaved transpose [d, rows]
        return np.ascontiguousarray(x.T[perm])

    half = L // 2
    n_pairs = n_heads // 3
    assert n_heads % 3 == 0 and n_pairs * 2 == 8, "sharding expects 12 heads/8 cores"

    in_maps = []
    core_heads = []
    for c in range(8):
        p, s = c // 2, c % 2
        headA = 3 * p if s == 0 else 3 * p + 2
        headB = 3 * p + 1
        qsl = slice(0, half) if s == 0 else slice(half, L)
        core_heads.append((headA, headB, qsl))

        cosq = np.ones((P, QT_N), dtype=np.float32)
        sinq = np.zeros((P, QT_N), dtype=np.float32)
        for (c0, tab) in ((0, slice(0, L)), (L, qsl)):
            ct, st = cos_t[tab].T, sin_t[tab].T
            w = ct.shape[1]
            cosq[0:H, c0:c0 + w] = ct
            cosq[H:P, c0:c0 + w] = ct
            sinq[0:H, c0:c0 + w] = -st
            sinq[H:P, c0:c0 + w] = st

        qt = np.zeros((P, QT_N), dtype=np.float32)
        qt[:, 0:L] = dei_T(q[0, :, headA, :])
        qt[:, L:QT_N] = dei_T(q[0, qsl, headB, :])

        im = {"qt": qt, "cosq": cosq, "sinq": sinq,
              "biasm": biasm, "ident": ident, "ones": onesm}
        for tag, h in (("a", headA), ("b", headB)):
            ktn = np.zeros((P, knew), dtype=np.float32)
            ktn[:, 0:L] = dei_T(k[0, :, h, :])
            im[f"kt{tag}"] = ktn
            im[f"kc{tag}"] = dei_T(kc_full[:, h, :]).astype(np.float16)
            vaa = np.zeros((n_kv_pad, d), dtype=np.float16)
            vaa[0:n_cache] = vc_full[:, h, :]
            vaa[n_cache:n_cache + L] = v[0, :, h, :]
            # pre-tile to the on-chip layout [P, KT*d]
            im[f"va{tag}"] = np.ascontiguousarray(
                vaa.reshape(n_kv_pad // P, P, d).transpose(1, 0, 2)
                .reshape(P, n_kv_pad))
        in_maps.append(im)

    res = run_bass_kernel_spmd(nc, in_maps, core_ids=list(range(8)))

    out = np.empty((B, L, n_heads, d), dtype=np.float32)
    for c in range(8):
        headA, headB, qsl = core_heads[c]
        o = res.results[c]["o"]
        out[0, :, headA, :] = o[0:L]
        out[0, qsl, headB, :] = o[L:L + half]
    return out
